# revision 44
# baseline (speedup 1.0000x reference)
"""AscendQwen3Attention (T=2048, HIDDEN=2048, HQ=32, HK=8, D=128) on 8 TRN2 cores.

Tensor-parallel over heads: core i owns q-heads [4i..4i+3] and kv-head i
(GQA rep=4 aligns exactly), w_qkv column-sharded to [2048, 768] per core,
w_o row-sharded to [512, 2048] per core. Each core computes a full [T, HIDDEN]
partial of the output projection (bf16); the host sums the 8 partials.

Single continuous PE stream per core, interleaving four kinds of segments so
the tensor engine never idles and stays at max p-state:
  [QKV tiles 0-3] [attn chunk0] [QKV 4-7] [op0 + attn1] [QKV 8-11]
  [op1 + attn2] [QKV 12-15] [op2 + attn3] [op3]

Per-tile QKV: 16+16 bf16 matmuls -> PSUM; ACT (copy) evicts q/k/v to bf16
SBUF; DVE computes per-head sum(x^2); ACT computes rsqrt via exp(-0.5*ln(v))
(Ln/Exp/Copy live in ONE activation table -> zero table reloads all kernel);
DVE does norm (x*istd*w) and rope, all bf16 (2-4x DVE modes); PE transposes
q/k heads to [d, t], deferred one tile so rope overlaps next tile's matmuls.

Attention per (chunk of 512 q, head): S^T[k,q] = K^T.T @ Q^T -> PSUM,
exp on ACT -> bf16 pt, 128-wide triangular mask on diagonal blocks only
(DVE). pt accumulates into a bf16 ptacc on DVE, so the denominator costs
ONE ones-matmul per (chunk, head) instead of one per key block (the den
matmul stream was ~15% of attention PE time). S issued in pairs with AV
pairs 4 blocks behind: the two AVs accumulate the same PSUM bank
back-to-back, which pipelines at stream rate with no bank-switch drain.
den matmul for head h-1 rides inside head h's stream (mm at pair 1,
recip+bcast at pair 2, at-mul after the AV drain) so its DVE/gpsimd
latency is hidden. Group-3 norm/rope runs as per-head filler closures
inside attn chunks 1-2 so the DVE queue never sees a multi-us blob ahead
of the latency-critical masks/adds.

Out-proj per chunk: 16 ho-tiles x 4 head matmuls in the pab pool (idle in
phase B -> no WAR against the attention S-pipeline), evicted on DVE,
DMA'd out as bf16 [2048, 2048] partials; the final chunk alternates two
DMA queues to shorten the tail.

PSUM budget (8 banks): ab(2: qkv A/B, late transposes, outproj) +
so(3: S pipeline + early transposes) + o(2: AV accum) + den(1).
"""

import os
import sys

sys.path.insert(0, "/opt/trn_rl_repo")

import numpy as np
from ml_dtypes import bfloat16

import concourse.bass as bass
import concourse.bacc as bacc
import concourse.tile as tile
import concourse.mybir as mybir
from concourse.bass_utils import run_bass_kernel_spmd

F32 = mybir.dt.float32
BF16 = mybir.dt.bfloat16
AF = mybir.ActivationFunctionType
ALU = mybir.AluOpType

T = 2048
HIDDEN = 2048
HQ, HK, D = 32, 8, 128
HALF = D // 2
MROPE = (16, 24, 24)
THETA = 1.0e6
EPS = 1e-6
N_CORES = 8
HQL = HQ // N_CORES            # 4 q heads per core
NH = HQL + 1                   # q heads + k head get rope/norm
FQKV = (HQL + 2) * D           # 768 qkv features per core
KT = HIDDEN // 128             # 16 contraction tiles
TT = T // 128                  # 16 token tiles
NQC = 4                        # q-chunks of 512
QCW = T // NQC                 # 512
SCALE = float(D) ** -0.5
CW = NH * HALF                 # 320 cos cols per t-tile

_CACHED = {}


def _build():
    nc = bacc.Bacc("TRN2", target_bir_lowering=False, debug=False,
                   num_devices=N_CORES)

    # ht packed tile-major: [128, (t_tile, kt, 128)]
    ht_d = nc.dram_tensor("ht", [128, TT * KT * 128], BF16, kind="ExternalInput")
    wqkv_d = nc.dram_tensor("wqkv", [128, KT * FQKV], BF16, kind="ExternalInput")
    wo_d = nc.dram_tensor("wo", [128, HQL * HIDDEN], BF16, kind="ExternalInput")
    # misc packed: cos | sin | qnw | knw | mask | ident
    MC = TT * CW
    MISC_N = 2 * MC + 4 * 128
    misc_d = nc.dram_tensor("misc", [128, MISC_N], BF16, kind="ExternalInput")
    out_d = nc.dram_tensor("out", [HIDDEN, T], BF16, kind="ExternalOutput")
    out_tiled = out_d.ap().rearrange("(a p) b -> a p b", p=128)

    with tile.TileContext(nc) as tc:
        with (
            tc.tile_pool(name="cst", bufs=1) as cst,
            tc.tile_pool(name="big", bufs=1) as big,
            tc.tile_pool(name="wrk", bufs=2) as wrk,
            tc.tile_pool(name="pab", bufs=2, space="PSUM") as pab,
            tc.tile_pool(name="pso", bufs=3, space="PSUM") as pso,
            tc.tile_pool(name="pacc", bufs=2, space="PSUM") as pacc,
            tc.tile_pool(name="pden", bufs=1, space="PSUM") as pden,
        ):
            # ---- persistent SBUF images -------------------------------------
            ht_sb = big.tile([128, TT * KT * 128], BF16, tag="ht")
            wqkv_sb = big.tile([128, KT * FQKV], BF16, tag="wqkv")
            wo_sb = big.tile([128, HQL * HIDDEN], BF16, tag="wo")
            misc_sb = cst.tile([128, MISC_N], BF16, tag="misc")
            COS0, SIN0 = 0, MC
            QNW0, KNW0 = 2 * MC, 2 * MC + 128
            MASK0, IDENT0 = 2 * MC + 256, 2 * MC + 384
            ones_sb = cst.tile([128, 1], BF16, tag="ones")
            eps_sb = cst.tile([128, 1], F32, tag="eps")
            # unified Q^T/K^T image: slot h<4 = q head h, slot 4 = k head
            qkt_sb = big.tile([128, NH * T], BF16, tag="qkt")
            v_sb = big.tile([128, T], BF16, tag="vsb")       # V   [t, d] tiled

            # Input DMAs spread over four parallel HW queues (sync/scalar/
            # vector/gpsimd) so the first QKV tile's deps (ht tile0 + wqkv
            # half A) land as early as possible.
            # ht in staged multi-tile chunks: 8-16KB per-partition rows give
            # ~4x the DMA rate of per-tile 4KB rows, and each chunk lands
            # comfortably before its first consumer tile.
            HB = KT * 128
            nc.sync.dma_start(ht_sb[:, 0:2 * HB], ht_d.ap()[:, 0:2 * HB])
            nc.scalar.dma_start(wqkv_sb[:, 0:8 * FQKV],
                                wqkv_d.ap()[:, 0:8 * FQKV])
            nc.gpsimd.dma_start(wqkv_sb[:, 8 * FQKV:KT * FQKV],
                                wqkv_d.ap()[:, 8 * FQKV:KT * FQKV])
            nc.sync.dma_start(ht_sb[:, 2 * HB:4 * HB],
                              ht_d.ap()[:, 2 * HB:4 * HB])
            nc.sync.dma_start(ht_sb[:, 4 * HB:7 * HB],
                              ht_d.ap()[:, 4 * HB:7 * HB])
            nc.scalar.dma_start(misc_sb[:], misc_d.ap())
            nc.gpsimd.dma_start(wo_sb[:], wo_d.ap())
            nc.sync.dma_start(ht_sb[:, 7 * HB:10 * HB],
                              ht_d.ap()[:, 7 * HB:10 * HB])
            nc.sync.dma_start(ht_sb[:, 10 * HB:13 * HB],
                              ht_d.ap()[:, 10 * HB:13 * HB])
            nc.sync.dma_start(ht_sb[:, 13 * HB:16 * HB],
                              ht_d.ap()[:, 13 * HB:16 * HB])
            nc.vector.memset(ones_sb[:], 1.0)
            nc.vector.memset(eps_sb[:], EPS)

            rots = {}

            KT_ORDER = list(range(KT))

            def qkv_mm_tile(t, xs, istd_g, li):
                """QKV matmuls + PSUM evict + per-head sum(x^2) for tile t.
                Accumulates ssq into istd_g[:, li*NH : (li+1)*NH]."""
                hcol = t * KT * 128
                psA = pab.tile([128, 512], F32, tag="ab", name=f"psA_{t}")
                for i, kt in enumerate(KT_ORDER):
                    nc.tensor.matmul(
                        psA[:], ht_sb[:, hcol + kt * 128: hcol + kt * 128 + 128],
                        wqkv_sb[:, kt * FQKV: kt * FQKV + 512],
                        start=(i == 0), stop=(i == KT - 1))
                xq = wrk.tile([128, 512], BF16, tag="xq", bufs=5, name=f"xq_{t}")
                nc.scalar.copy(xq[:], psA[:])
                psB = pab.tile([128, 512], F32, tag="ab", name=f"psB_{t}")
                for i, kt in enumerate(KT_ORDER):
                    nc.tensor.matmul(
                        psB[:, 0:256],
                        ht_sb[:, hcol + kt * 128: hcol + kt * 128 + 128],
                        wqkv_sb[:, kt * FQKV + 512: kt * FQKV + 768],
                        start=(i == 0), stop=(i == KT - 1))
                xk = wrk.tile([128, 128], BF16, tag="xk", bufs=5, name=f"xk_{t}")
                nc.scalar.copy(xk[:], psB[:, 0:128])
                nc.scalar.copy(v_sb[:, t * 128:(t + 1) * 128], psB[:, 128:256])
                # sum(x^2) on ACT (Square + accumulator, same table as Exp)
                sq = wrk.tile([128, 128], BF16, tag="sq", name=f"sq_{t}")
                for h in range(NH):
                    xh = xq[:, h * 128:(h + 1) * 128] if h < HQL else xk[:]
                    nc.scalar.activation(sq[:], xh, AF.Square,
                                         accum_out=istd_g[:, li * NH + h:
                                                          li * NH + h + 1])
                xs[t] = (xq, xk)

            def group_rsqrt(istd_g, G):
                """In-place istd_g <- rsqrt(istd_g/D + eps), batched over a
                4-tile group ([128, 20] ops). Newton from constant seed: v is
                mean(x^2) of the qkv projection, concentrated around ~0.8."""
                W = 4 * NH
                v = wrk.tile([128, W], F32, tag="nv", name=f"nv_{G}")
                nc.vector.tensor_scalar(v[:], istd_g[:, 0:W], 1.0 / D, EPS,
                                        op0=ALU.mult, op1=ALU.add)
                Y0 = 1.1
                # y1 = 1.5*Y0 - 0.5*Y0^3 * v  (first Newton step is linear in v)
                nc.vector.tensor_scalar(istd_g[:, 0:W], v[:],
                                        -0.5 * Y0 ** 3, 1.5 * Y0,
                                        op0=ALU.mult, op1=ALU.add)
                a = wrk.tile([128, W], F32, tag="na", name=f"na_{G}")
                for it in range(3):
                    y = istd_g[:, 0:W]
                    nc.vector.tensor_mul(a[:], y, y)
                    nc.vector.tensor_mul(a[:], a[:], v[:])
                    nc.vector.tensor_scalar(a[:], a[:], -0.5, 1.5,
                                            op0=ALU.mult, op1=ALU.add)
                    nc.vector.tensor_mul(y, y, a[:])

            PW = 2 * CW  # 640: one tile-pair's worth of (tile, head, half) cols

            def _vp_norm(t0, xs, istd_g, li0, li01, st):
                if 'xn' not in st:
                    st['xn'] = wrk.tile([128, 2 * PW], BF16, tag="xn",
                                        name=f"xn_{t0}")
                xnv = st['xn'][:].rearrange("p (s r) -> p s r", s=2)
                xq, xk = xs.pop(t0 + li01)
                for h in range(NH):
                    xh = (xq[:, h * 128:(h + 1) * 128] if h < HQL
                          else xk[:])
                    w0 = QNW0 if h < HQL else KNW0
                    c0 = li01 * CW + h * HALF
                    nc.vector.scalar_tensor_tensor(
                        xnv[:, :, c0:c0 + HALF],
                        xh.rearrange("p (s d) -> p s d", s=2),
                        istd_g[:, (li0 + li01) * NH + h:
                               (li0 + li01) * NH + h + 1],
                        misc_sb[:, w0:w0 + 128].rearrange(
                            "p (s d) -> p s d", s=2),
                        op0=ALU.mult, op1=ALU.mult)

            def _vp_rope1(t0, st):
                rot = wrk.tile([128, 2 * PW], BF16, tag="rot", bufs=4,
                               name=f"rot_{t0}")
                st['rot'] = rot
                xn = st['xn']
                x1 = xn[:, 0:PW]
                x2 = xn[:, PW:2 * PW]
                cp = misc_sb[:, COS0 + t0 * CW:COS0 + t0 * CW + PW]
                sp = misc_sb[:, SIN0 + t0 * CW:SIN0 + t0 * CW + PW]
                ta = wrk.tile([128, PW], BF16, tag="ta", name=f"ta_{t0}")
                tb = wrk.tile([128, PW], BF16, tag="tb", name=f"tb_{t0}")
                nc.vector.tensor_mul(ta[:], x1, cp)
                nc.vector.tensor_mul(tb[:], x2, sp)
                nc.vector.tensor_sub(rot[:, 0:PW], ta[:], tb[:])

            def _vp_rope2(t0, st):
                rot = st['rot']
                xn = st['xn']
                x1 = xn[:, 0:PW]
                x2 = xn[:, PW:2 * PW]
                cp = misc_sb[:, COS0 + t0 * CW:COS0 + t0 * CW + PW]
                sp = misc_sb[:, SIN0 + t0 * CW:SIN0 + t0 * CW + PW]
                tc2 = wrk.tile([128, PW], BF16, tag="ta", name=f"tc_{t0}")
                td = wrk.tile([128, PW], BF16, tag="tb", name=f"td_{t0}")
                nc.vector.tensor_mul(tc2[:], x2, cp)
                nc.vector.tensor_mul(td[:], x1, sp)
                nc.vector.tensor_add(rot[:, PW:2 * PW], tc2[:], td[:])
                rots[t0] = rot
                rots[t0 + 1] = rot

            def qkv_vec_pair(t0, xs, istd_g, li0):
                """norm + rope for tiles t0, t0+1 (all DVE). Half-major pair
                layout: xn/rot [128, (half, tile2, head, 64)] so the 6 rope
                multiplies are contiguous [128, 640] ops."""
                st = {}
                _vp_norm(t0, xs, istd_g, li0, 0, st)
                _vp_norm(t0, xs, istd_g, li0, 1, st)
                _vp_rope1(t0, st)
                _vp_rope2(t0, st)

            def vec_pair_fillers(t0, xs, istd_g, li0):
                """qkv_vec_pair split into 4 closures, issued one per
                attention head so the DVE queue never sees a multi-us rope
                blob ahead of the latency-critical mask/ptacc adds."""
                st = {}
                return [
                    lambda: _vp_norm(t0, xs, istd_g, li0, 0, st),
                    lambda: _vp_norm(t0, xs, istd_g, li0, 1, st),
                    lambda: _vp_rope1(t0, st),
                    lambda: _vp_rope2(t0, st),
                ]

            def qkv_transposes(t, pool=None, tag=None):
                rot = rots.pop(t)
                li01 = t % 2
                # phase A: pso pool (idle there, so transposes never serialize
                # the qkv psum recycling); phase B: pab (idle there) so the
                # S-pipeline's pso ring stays private to attention
                pool = pool or pso
                tp = pool.tile([128, NH * 128], BF16, tag=tag or "so",
                               name=f"tp_{t}")
                ident = misc_sb[:, IDENT0:IDENT0 + 128]
                for h in range(NH):
                    c0 = li01 * CW + h * HALF
                    # two half-width transposes (PE weights APs must be 2D):
                    # rot half s lands on psum partitions [64s, 64s+64)
                    nc.tensor.transpose(
                        tp[0:64, h * 128:(h + 1) * 128],
                        rot[:, c0:c0 + HALF], ident, tile_position=(0, 0))
                    nc.tensor.transpose(
                        tp[64:128, h * 128:(h + 1) * 128],
                        rot[:, PW + c0:PW + c0 + HALF], ident,
                        tile_position=(0, 64))
                # one wide strided copy: slot h goes to qkt col h*T + t*128
                dst = qkt_sb[:].rearrange("p (h t) -> p h t", h=NH)[
                    :, :, t * 128:(t + 1) * 128]
                src = tp[:].rearrange("p (h d) -> p h d", h=NH)
                nc.vector.tensor_copy(dst, src)

            at_tiles = {}
            den_acc = {}
            den_ps_t = {}

            def _issue_den(g, h):
                """PE: one 512-col den matmul over the DVE-accumulated ptacc
                (replaces a per-key-block ones-matmul stream)."""
                ptacc = den_acc.pop((g, h))
                den_ps = pden.tile([1, QCW], F32, tag="den",
                                   name=f"den_{g}_{h}")
                nc.tensor.matmul(den_ps[0:1, :], ones_sb[:, 0:1], ptacc[:],
                                 start=True, stop=True)
                den_ps_t[(g, h)] = den_ps

            def _den_post(g, h):
                """DVE reciprocal + gpsimd partition broadcast for head h."""
                den_ps = den_ps_t.pop((g, h))
                den_r = wrk.tile([1, QCW], F32, tag="denr",
                                 name=f"denr_{g}_{h}")
                nc.vector.reciprocal_approx_fast(den_r[0:1, :],
                                                 den_ps[0:1, :])
                den_b = wrk.tile([128, QCW], F32, tag="denb",
                                 name=f"denb_{g}_{h}")
                nc.gpsimd.partition_broadcast(den_b[:], den_r[0:1, :])
                at, o_ps, _ = at_tiles[(g, h)]
                at_tiles[(g, h)] = (at, o_ps, den_b)

            def attn_chunk(g, fillers=()):
                """Causal attention for q-chunk g (512 q), heads sequential,
                S issued 3 key-blocks ahead of AV. den for head h-1 is
                computed inside head h's stream (matmul at kb2, recip+bcast
                at kb4, at-mul after the AV drain). fillers: one closure per
                head of deferred phase-A DVE work, spread between heads."""
                nkb = 4 * g + 4
                for h in range(HQL):
                    o_ps = pacc.tile([128, QCW], F32, tag="o", name=f"o_{g}_{h}")
                    ptacc = wrk.tile([128, QCW], BF16, tag="ptacc", bufs=2,
                                     name=f"ptacc_{g}_{h}")
                    pend = []
                    pt0 = [None]

                    def issue_s(kb):
                        r = kb - 4 * g
                        q0 = 128 * r if r > 0 else 0
                        s_ps = pso.tile([128, QCW], F32, tag="so",
                                        name=f"s_{g}_{h}_{kb}")
                        nc.tensor.matmul(
                            s_ps[:, q0:QCW],
                            qkt_sb[:, HQL * T + kb * 128:
                                   HQL * T + (kb + 1) * 128],
                            qkt_sb[:, h * T + g * QCW + q0:
                                   h * T + (g + 1) * QCW],
                            start=True, stop=True)
                        pt = wrk.tile([128, QCW], BF16, tag="pt", bufs=8,
                                      name=f"pt_{g}_{h}_{kb}")
                        nc.scalar.activation(pt[:, q0:QCW], s_ps[:, q0:QCW],
                                             AF.Exp, scale=SCALE)
                        if r >= 0:
                            nc.vector.tensor_mul(
                                pt[:, q0:q0 + 128], pt[:, q0:q0 + 128],
                                misc_sb[:, MASK0:MASK0 + 128])
                        if kb == 0:
                            if g == 0:
                                nc.vector.tensor_copy(ptacc[:], pt[:])
                            else:
                                pt0[0] = pt
                        elif pt0[0] is not None:
                            nc.vector.tensor_add(ptacc[:], pt0[0][:], pt[:])
                            pt0[0] = None
                        else:
                            nc.vector.tensor_add(ptacc[:, q0:QCW],
                                                 ptacc[:, q0:QCW],
                                                 pt[:, q0:QCW])
                        pend.append((kb, pt, q0))

                    def issue_av():
                        kb, pt, q0 = pend.pop(0)
                        nc.tensor.matmul(o_ps[:, q0:QCW],
                                         v_sb[:, kb * 128:(kb + 1) * 128],
                                         pt[:, q0:QCW], start=(kb == 0),
                                         stop=(kb == nkb - 1),
                                         skip_group_check=True)

                    # S issued in pairs, AV consumed in pairs 4 behind: the
                    # two AVs accumulate the same PSUM bank back-to-back,
                    # which pipelines without a bank-switch drain
                    den_kb2 = 2
                    post_kb2 = 4 if nkb > 4 else 2
                    for kb2 in range(0, nkb, 2):
                        issue_s(kb2)
                        issue_s(kb2 + 1)
                        if h > 0:
                            if kb2 == den_kb2:
                                _issue_den(g, h - 1)
                            if kb2 == post_kb2:
                                _den_post(g, h - 1)
                        if kb2 >= 4:
                            issue_av()
                            issue_av()
                    while pend:
                        issue_av()

                    at = wrk.tile([128, QCW], BF16, tag="at", bufs=8,
                                  name=f"at_{g}_{h}")
                    # deferred: at-mul for head h-1 issued during head h's
                    # stream so the gpsimd broadcast latency is hidden
                    if h > 0:
                        _flush_at(g, h - 1)
                    at_tiles[(g, h)] = (at, o_ps, None)
                    den_acc[(g, h)] = ptacc
                    if h < len(fillers):
                        fillers[h]()
                _issue_den(g, HQL - 1)
                _den_post(g, HQL - 1)
                _flush_at(g, HQL - 1)

            def _flush_at(g, h):
                at, o_ps, den_b = at_tiles[(g, h)]
                if o_ps is not None:
                    nc.vector.tensor_mul(at[:], o_ps[:], den_b[:])
                    at_tiles[(g, h)] = (at, None, None)

            def outproj_parts(g, nparts=4, evict_eng="dve"):
                # op tiles live in the pab pool: it is idle in phase B, so
                # outproj never contends with the attention S-pipeline's pso
                # ring (whose reuse waits on exp evictions)
                st = {'prev': None, 'pho': None}

                def part(lo, hi):
                    def run():
                        for ho in range(lo, hi):
                            op = pab.tile([128, QCW], F32, tag="ab",
                                          name=f"op_{g}_{ho}")
                            for f in range(HQL):
                                nc.tensor.matmul(
                                    op[:],
                                    wo_sb[:, f * HIDDEN + ho * 128:
                                          f * HIDDEN + ho * 128 + 128],
                                    at_tiles[(g, f)][0][:],
                                    start=(f == 0), stop=(f == HQL - 1))
                            if st['prev'] is not None:
                                _evict_osb(g, st['pho'], st['prev'],
                                           evict_eng)
                            st['prev'] = op
                            st['pho'] = ho
                        if hi == TT:
                            _evict_osb(g, TT - 1, st['prev'], evict_eng)
                    return run

                step = TT // nparts
                return [part(i * step, (i + 1) * step) for i in range(nparts)]

            def outproj(g):
                for p in outproj_parts(g, 1):
                    p()

            def _evict_osb(g, ho, op, eng="dve"):
                osb = wrk.tile([128, QCW], BF16, tag="osb", bufs=4,
                               name=f"osb_{g}_{ho}")
                # standalone outproj: DVE (keeps ACT pure-exp near attention)
                # filler-mode (inside attn3): ACT, because the DVE add chain
                # would delay the eviction and stall the pab ring
                if eng == "act":
                    nc.scalar.copy(osb[:], op[:])
                else:
                    nc.vector.tensor_copy(osb[:], op[:])
                # final chunk drains its output over two parallel queues so
                # the last eviction's DMA isn't queued behind 15 others
                eng = nc.gpsimd if (g == NQC - 1 and ho % 2 == 1) else nc.sync
                eng.dma_start(
                    out_tiled[ho][:, g * QCW:(g + 1) * QCW], osb[:])

            # ---- main schedule: single ACT table (Copy/Square/Exp) ---------
            # Phase A in groups of 4 tiles: [matmuls+evicts+ssq x4] ->
            # [batched Newton rsqrt] -> [norm+rope x4]; transposes of group G
            # are issued inside group G+1's matmul window. Group 3's vector
            # work is interleaved with the first attention chunks so it never
            # head-of-line-blocks the DVE queue at the phase boundary.
            xs = {}
            istd_gs = []
            for G in range(4):
                istd_g = wrk.tile([128, 4 * NH], F32, tag="istdg",
                                  name=f"istdg_{G}")
                istd_gs.append(istd_g)
                for li, t in enumerate(range(4 * G, 4 * G + 4)):
                    qkv_mm_tile(t, xs, istd_g, li)
                    if t - 4 in rots:
                        qkv_transposes(t - 4)
                group_rsqrt(istd_g, G)
                if G < 3:
                    qkv_vec_pair(4 * G, xs, istd_g, 0)
                    qkv_vec_pair(4 * G + 2, xs, istd_g, 2)
            # Attention (chunk g needs only q/k tiles <= 4g+3) interleaved
            # with group 3's deferred vector work (as per-head fillers) and
            # out-projections.
            attn_chunk(0)
            attn_chunk(1, fillers=vec_pair_fillers(12, xs, istd_gs[3], 0))
            outproj(0)
            attn_chunk(2, fillers=vec_pair_fillers(14, xs, istd_gs[3], 2))
            for t in range(12, 16):
                qkv_transposes(t, pool=pab, tag="ab")
            outproj(1)
            attn_chunk(3)
            outproj(2)
            outproj(NQC - 1)

    nc.compile()
    return nc


def _pack_rows(a):
    """[N*128, M] -> [128, N*M] SBUF image (partition-major k-tiles)."""
    n = a.shape[0] // 128
    return np.ascontiguousarray(
        a.reshape(n, 128, a.shape[1]).transpose(1, 0, 2).reshape(128, -1))


def _pack_ht(hsT):
    """hidden^T [2048, 2048] -> [128, (t_tile, kt, 128)] bf16."""
    # hsT[kt*128+p, tile*128+c] -> img[p, ((tile*KT)+kt)*128 + c]
    a = hsT.reshape(KT, 128, TT, 128)          # [kt, p, tile, c]
    a = a.transpose(1, 2, 0, 3)                # [p, tile, kt, c]
    return np.ascontiguousarray(a.reshape(128, -1))


def _cos_sin(positions):
    j = np.arange(HALF, dtype=np.float32)
    inv_freq = (np.float32(THETA) ** (-j / np.float32(HALF))).astype(np.float32)
    pos = positions.astype(np.float32)
    freqs3 = pos[:, :, None] * inv_freq[None, None, :]      # [3, T, HALF] f32
    sel = np.zeros(HALF, dtype=np.int64)
    sel[MROPE[0]:MROPE[0] + MROPE[1]] = 1
    sel[MROPE[0] + MROPE[1]:] = 2
    freqs = freqs3[sel, :, np.arange(HALF)].T               # [T, HALF]
    freqs = np.ascontiguousarray(freqs.astype(np.float32))
    return np.cos(freqs).astype(np.float32), np.sin(freqs).astype(np.float32)


def _prep_inputs(hidden_states, positions, w_qkv, w_o, q_norm_w, k_norm_w):
    ht = _pack_ht(np.ascontiguousarray(hidden_states.T).astype(bfloat16))
    cos, sin = _cos_sin(positions)
    cos_p = _pack_rows(np.tile(cos, (1, NH)).astype(bfloat16))
    sin_p = _pack_rows(np.tile(sin, (1, NH)).astype(bfloat16))
    qnw = np.tile(np.asarray(q_norm_w, np.float32)[None, :], (128, 1)
                  ).astype(bfloat16)
    knw = np.tile(np.asarray(k_norm_w, np.float32)[None, :], (128, 1)
                  ).astype(bfloat16)
    # diag mask: keys on partitions p, q offset j in the 128-wide diag block:
    # visible iff j >= p
    mask = (np.arange(128)[None, :] >= np.arange(128)[:, None]).astype(bfloat16)
    ident = np.eye(128, dtype=bfloat16)
    misc = np.concatenate([cos_p, sin_p, qnw, knw, mask, ident],
                          axis=1).astype(bfloat16)
    misc = np.ascontiguousarray(misc)

    in_maps = []
    for i in range(N_CORES):
        q0 = HQL * i * D
        wq = w_qkv[:, q0: q0 + HQL * D]
        wk = w_qkv[:, HQ * D + i * D: HQ * D + (i + 1) * D]
        wv = w_qkv[:, (HQ + HK) * D + i * D: (HQ + HK) * D + (i + 1) * D]
        wqkv_i = np.concatenate([wq, wk, wv], axis=1).astype(bfloat16)
        wo_i = w_o[HQL * i * D: HQL * (i + 1) * D, :].astype(bfloat16)
        in_maps.append({
            "ht": ht,
            "wqkv": _pack_rows(wqkv_i),
            "wo": _pack_rows(wo_i),
            "misc": misc,
        })
    return in_maps


LAST_RESULTS = None


def kernel(**inputs):
    global LAST_RESULTS
    if "nc" not in _CACHED:
        _CACHED["nc"] = _build()
    nc = _CACHED["nc"]
    in_maps = _prep_inputs(**{k: np.asarray(v) for k, v in inputs.items()})
    trace = bool(os.environ.get("BASS_TRACE"))
    res = run_bass_kernel_spmd(nc, in_maps, core_ids=list(range(N_CORES)),
                               trace=trace)
    LAST_RESULTS = res
    acc = np.zeros((HIDDEN, T), dtype=np.float32)
    for i in range(N_CORES):
        acc += res.results[i]["out"].astype(np.float32)
    return np.ascontiguousarray(acc.T)



# revision 47
# speedup vs baseline: 1.1792x; 1.1792x over previous
"""AscendQwen3Attention (T=2048, HIDDEN=2048, HQ=32, HK=8, D=128) on 8 TRN2 cores.

Tensor-parallel over heads: core i owns q-heads [4i..4i+3] and kv-head i
(GQA rep=4 aligns exactly), w_qkv column-sharded to [2048, 768] per core,
w_o row-sharded to [512, 2048] per core. Each core computes a full [T, HIDDEN]
partial of the output projection (bf16); the host sums the 8 partials.

Single continuous PE stream per core, interleaving four kinds of segments so
the tensor engine never idles and stays at max p-state:
  [QKV tiles 0-3] [attn chunk0] [QKV 4-7] [op0 + attn1] [QKV 8-11]
  [op1 + attn2] [QKV 12-15] [op2 + attn3] [op3]

Per-tile QKV: 16+16 bf16 matmuls -> PSUM; ACT (copy) evicts q/k/v to bf16
SBUF; DVE computes per-head sum(x^2); ACT computes rsqrt via exp(-0.5*ln(v))
(Ln/Exp/Copy live in ONE activation table -> zero table reloads all kernel);
DVE does norm (x*istd*w) and rope, all bf16 (2-4x DVE modes); PE transposes
q/k heads to [d, t], deferred one tile so rope overlaps next tile's matmuls.

Attention per (chunk of 512 q, head): S^T[k,q] = K^T.T @ Q^T -> PSUM,
exp on ACT -> bf16 pt, 128-wide triangular mask on diagonal blocks only
(DVE). pt accumulates into a bf16 ptacc on DVE, so the denominator costs
ONE ones-matmul per (chunk, head) instead of one per key block (the den
matmul stream was ~15% of attention PE time). S issued in pairs with AV
pairs 4 blocks behind: the two AVs accumulate the same PSUM bank
back-to-back, which pipelines at stream rate with no bank-switch drain.
den matmul for head h-1 rides inside head h's stream (mm at pair 1,
recip+bcast at pair 2, at-mul after the AV drain) so its DVE/gpsimd
latency is hidden. Group-3 norm/rope runs as per-head filler closures
inside attn chunks 1-2 so the DVE queue never sees a multi-us blob ahead
of the latency-critical masks/adds.

Out-proj per chunk: 16 ho-tiles x 4 head matmuls in the pab pool (idle in
phase B -> no WAR against the attention S-pipeline), evicted on DVE,
DMA'd out as bf16 [2048, 2048] partials; the final chunk alternates two
DMA queues to shorten the tail.

PSUM budget (8 banks): ab(2: qkv A/B, late transposes, outproj) +
so(3: S pipeline + early transposes) + o(2: AV accum) + den(1).
"""

import os
import sys

sys.path.insert(0, "/opt/trn_rl_repo")

import numpy as np
from ml_dtypes import bfloat16

import concourse.bass as bass
import concourse.bacc as bacc
import concourse.tile as tile
import concourse.mybir as mybir
from concourse.bass_utils import run_bass_kernel_spmd

F32 = mybir.dt.float32
BF16 = mybir.dt.bfloat16
AF = mybir.ActivationFunctionType
ALU = mybir.AluOpType

T = 2048
HIDDEN = 2048
HQ, HK, D = 32, 8, 128
HALF = D // 2
MROPE = (16, 24, 24)
THETA = 1.0e6
EPS = 1e-6
N_CORES = 8
HQL = HQ // N_CORES            # 4 q heads per core
NH = HQL + 1                   # q heads + k head get rope/norm
FQKV = (HQL + 2) * D           # 768 qkv features per core
KT = HIDDEN // 128             # 16 contraction tiles
TT = T // 128                  # 16 token tiles
NQC = 4                        # q-chunks of 512
QCW = T // NQC                 # 512
SCALE = float(D) ** -0.5
CW = NH * HALF                 # 320 cos cols per t-tile

_CACHED = {}


def _build():
    nc = bacc.Bacc("TRN2", target_bir_lowering=False, debug=False,
                   num_devices=N_CORES)

    # ht packed tile-major: [128, (t_tile, kt, 128)]
    ht_d = nc.dram_tensor("ht", [128, TT * KT * 128], BF16, kind="ExternalInput")
    wqkv_d = nc.dram_tensor("wqkv", [128, KT * FQKV], BF16, kind="ExternalInput")
    wo_d = nc.dram_tensor("wo", [128, HQL * HIDDEN], BF16, kind="ExternalInput")
    # misc packed: cos | sin | qnw | knw | mask | ident
    MC = TT * CW
    MISC_N = 2 * MC + 4 * 128
    misc_d = nc.dram_tensor("misc", [128, MISC_N], BF16, kind="ExternalInput")
    out_d = nc.dram_tensor("out", [HIDDEN, T], BF16, kind="ExternalOutput")
    out_tiled = out_d.ap().rearrange("(a p) b -> a p b", p=128)

    with tile.TileContext(nc) as tc:
        with (
            tc.tile_pool(name="cst", bufs=1) as cst,
            tc.tile_pool(name="big", bufs=1) as big,
            tc.tile_pool(name="wrk", bufs=2) as wrk,
            tc.tile_pool(name="pab", bufs=2, space="PSUM") as pab,
            tc.tile_pool(name="pso", bufs=3, space="PSUM") as pso,
            tc.tile_pool(name="pacc", bufs=2, space="PSUM") as pacc,
            tc.tile_pool(name="pden", bufs=1, space="PSUM") as pden,
        ):
            # ---- persistent SBUF images -------------------------------------
            ht_sb = big.tile([128, TT * KT * 128], BF16, tag="ht")
            wqkv_sb = big.tile([128, KT * FQKV], BF16, tag="wqkv")
            wo_sb = big.tile([128, HQL * HIDDEN], BF16, tag="wo")
            misc_sb = cst.tile([128, MISC_N], BF16, tag="misc")
            COS0, SIN0 = 0, MC
            QNW0, KNW0 = 2 * MC, 2 * MC + 128
            MASK0, IDENT0 = 2 * MC + 256, 2 * MC + 384
            ones_sb = cst.tile([128, 1], BF16, tag="ones")
            eps_sb = cst.tile([128, 1], F32, tag="eps")
            # unified Q^T/K^T image: slot h<4 = q head h, slot 4 = k head
            qkt_sb = big.tile([128, NH * T], BF16, tag="qkt")
            v_sb = big.tile([128, T], BF16, tag="vsb")       # V   [t, d] tiled

            # Input DMAs spread over four parallel HW queues (sync/scalar/
            # vector/gpsimd) so the first QKV tile's deps (ht tile0 + wqkv
            # half A) land as early as possible.
            # ht in staged multi-tile chunks: 8-16KB per-partition rows give
            # ~4x the DMA rate of per-tile 4KB rows, and each chunk lands
            # comfortably before its first consumer tile.
            HB = KT * 128
            # wqkv quarters: each queue only has to deliver its FIRST quarter
            # early (the interleaved KT_ORDER consumes q1-scalar, q1-gpsimd,
            # q2-scalar, q2-gpsimd), halving exposure to a slow queue
            nc.sync.dma_start(ht_sb[:, 0:2 * HB], ht_d.ap()[:, 0:2 * HB])
            nc.scalar.dma_start(wqkv_sb[:, 0:4 * FQKV],
                                wqkv_d.ap()[:, 0:4 * FQKV])
            nc.gpsimd.dma_start(wqkv_sb[:, 8 * FQKV:12 * FQKV],
                                wqkv_d.ap()[:, 8 * FQKV:12 * FQKV])
            nc.scalar.dma_start(wqkv_sb[:, 4 * FQKV:8 * FQKV],
                                wqkv_d.ap()[:, 4 * FQKV:8 * FQKV])
            nc.gpsimd.dma_start(wqkv_sb[:, 12 * FQKV:KT * FQKV],
                                wqkv_d.ap()[:, 12 * FQKV:KT * FQKV])
            nc.sync.dma_start(ht_sb[:, 2 * HB:4 * HB],
                              ht_d.ap()[:, 2 * HB:4 * HB])
            nc.sync.dma_start(ht_sb[:, 4 * HB:7 * HB],
                              ht_d.ap()[:, 4 * HB:7 * HB])
            nc.scalar.dma_start(misc_sb[:], misc_d.ap())
            nc.gpsimd.dma_start(wo_sb[:], wo_d.ap())
            nc.sync.dma_start(ht_sb[:, 7 * HB:10 * HB],
                              ht_d.ap()[:, 7 * HB:10 * HB])
            nc.sync.dma_start(ht_sb[:, 10 * HB:13 * HB],
                              ht_d.ap()[:, 10 * HB:13 * HB])
            nc.sync.dma_start(ht_sb[:, 13 * HB:16 * HB],
                              ht_d.ap()[:, 13 * HB:16 * HB])
            nc.vector.memset(ones_sb[:], 1.0)
            nc.vector.memset(eps_sb[:], EPS)

            rots = {}

            # consume wqkv quarters in DMA-arrival order: scalar-q1,
            # gpsimd-q1, scalar-q2, gpsimd-q2
            KT_ORDER = [0, 1, 2, 3, 8, 9, 10, 11, 4, 5, 6, 7, 12, 13, 14, 15]

            def qkv_mm_tile(t, xs, istd_g, li):
                """QKV matmuls + PSUM evict + per-head sum(x^2) for tile t.
                Accumulates ssq into istd_g[:, li*NH : (li+1)*NH]."""
                hcol = t * KT * 128
                psA = pab.tile([128, 512], F32, tag="ab", name=f"psA_{t}")
                for i, kt in enumerate(KT_ORDER):
                    nc.tensor.matmul(
                        psA[:], ht_sb[:, hcol + kt * 128: hcol + kt * 128 + 128],
                        wqkv_sb[:, kt * FQKV: kt * FQKV + 512],
                        start=(i == 0), stop=(i == KT - 1))
                xq = wrk.tile([128, 512], BF16, tag="xq", bufs=5, name=f"xq_{t}")
                nc.scalar.copy(xq[:], psA[:])
                psB = pab.tile([128, 512], F32, tag="ab", name=f"psB_{t}")
                for i, kt in enumerate(KT_ORDER):
                    nc.tensor.matmul(
                        psB[:, 0:256],
                        ht_sb[:, hcol + kt * 128: hcol + kt * 128 + 128],
                        wqkv_sb[:, kt * FQKV + 512: kt * FQKV + 768],
                        start=(i == 0), stop=(i == KT - 1))
                xk = wrk.tile([128, 128], BF16, tag="xk", bufs=5, name=f"xk_{t}")
                nc.scalar.copy(xk[:], psB[:, 0:128])
                nc.scalar.copy(v_sb[:, t * 128:(t + 1) * 128], psB[:, 128:256])
                # sum(x^2) on ACT (Square + accumulator, same table as Exp)
                sq = wrk.tile([128, 128], BF16, tag="sq", name=f"sq_{t}")
                for h in range(NH):
                    xh = xq[:, h * 128:(h + 1) * 128] if h < HQL else xk[:]
                    nc.scalar.activation(sq[:], xh, AF.Square,
                                         accum_out=istd_g[:, li * NH + h:
                                                          li * NH + h + 1])
                xs[t] = (xq, xk)

            def group_rsqrt(istd_g, G):
                """In-place istd_g <- rsqrt(istd_g/D + eps), batched over a
                4-tile group ([128, 20] ops). Newton from constant seed: v is
                mean(x^2) of the qkv projection, concentrated around ~0.8."""
                W = 4 * NH
                v = wrk.tile([128, W], F32, tag="nv", name=f"nv_{G}")
                nc.vector.tensor_scalar(v[:], istd_g[:, 0:W], 1.0 / D, EPS,
                                        op0=ALU.mult, op1=ALU.add)
                Y0 = 1.1
                # y1 = 1.5*Y0 - 0.5*Y0^3 * v  (first Newton step is linear in v)
                nc.vector.tensor_scalar(istd_g[:, 0:W], v[:],
                                        -0.5 * Y0 ** 3, 1.5 * Y0,
                                        op0=ALU.mult, op1=ALU.add)
                a = wrk.tile([128, W], F32, tag="na", name=f"na_{G}")
                for it in range(3):
                    y = istd_g[:, 0:W]
                    nc.vector.tensor_mul(a[:], y, y)
                    nc.vector.tensor_mul(a[:], a[:], v[:])
                    nc.vector.tensor_scalar(a[:], a[:], -0.5, 1.5,
                                            op0=ALU.mult, op1=ALU.add)
                    nc.vector.tensor_mul(y, y, a[:])

            PW = 2 * CW  # 640: one tile-pair's worth of (tile, head, half) cols

            def _vp_norm(t0, xs, istd_g, li0, li01, st):
                if 'xn' not in st:
                    st['xn'] = wrk.tile([128, 2 * PW], BF16, tag="xn",
                                        name=f"xn_{t0}")
                xnv = st['xn'][:].rearrange("p (s r) -> p s r", s=2)
                xq, xk = xs.pop(t0 + li01)
                for h in range(NH):
                    xh = (xq[:, h * 128:(h + 1) * 128] if h < HQL
                          else xk[:])
                    w0 = QNW0 if h < HQL else KNW0
                    c0 = li01 * CW + h * HALF
                    nc.vector.scalar_tensor_tensor(
                        xnv[:, :, c0:c0 + HALF],
                        xh.rearrange("p (s d) -> p s d", s=2),
                        istd_g[:, (li0 + li01) * NH + h:
                               (li0 + li01) * NH + h + 1],
                        misc_sb[:, w0:w0 + 128].rearrange(
                            "p (s d) -> p s d", s=2),
                        op0=ALU.mult, op1=ALU.mult)

            def _vp_rope1(t0, st):
                rot = wrk.tile([128, 2 * PW], BF16, tag="rot", bufs=4,
                               name=f"rot_{t0}")
                st['rot'] = rot
                xn = st['xn']
                x1 = xn[:, 0:PW]
                x2 = xn[:, PW:2 * PW]
                cp = misc_sb[:, COS0 + t0 * CW:COS0 + t0 * CW + PW]
                sp = misc_sb[:, SIN0 + t0 * CW:SIN0 + t0 * CW + PW]
                ta = wrk.tile([128, PW], BF16, tag="ta", name=f"ta_{t0}")
                tb = wrk.tile([128, PW], BF16, tag="tb", name=f"tb_{t0}")
                nc.vector.tensor_mul(ta[:], x1, cp)
                nc.vector.tensor_mul(tb[:], x2, sp)
                nc.vector.tensor_sub(rot[:, 0:PW], ta[:], tb[:])

            def _vp_rope2(t0, st):
                rot = st['rot']
                xn = st['xn']
                x1 = xn[:, 0:PW]
                x2 = xn[:, PW:2 * PW]
                cp = misc_sb[:, COS0 + t0 * CW:COS0 + t0 * CW + PW]
                sp = misc_sb[:, SIN0 + t0 * CW:SIN0 + t0 * CW + PW]
                tc2 = wrk.tile([128, PW], BF16, tag="ta", name=f"tc_{t0}")
                td = wrk.tile([128, PW], BF16, tag="tb", name=f"td_{t0}")
                nc.vector.tensor_mul(tc2[:], x2, cp)
                nc.vector.tensor_mul(td[:], x1, sp)
                nc.vector.tensor_add(rot[:, PW:2 * PW], tc2[:], td[:])
                rots[t0] = rot
                rots[t0 + 1] = rot

            def qkv_vec_pair(t0, xs, istd_g, li0):
                """norm + rope for tiles t0, t0+1 (all DVE). Half-major pair
                layout: xn/rot [128, (half, tile2, head, 64)] so the 6 rope
                multiplies are contiguous [128, 640] ops."""
                st = {}
                _vp_norm(t0, xs, istd_g, li0, 0, st)
                _vp_norm(t0, xs, istd_g, li0, 1, st)
                _vp_rope1(t0, st)
                _vp_rope2(t0, st)

            def vec_pair_fillers(t0, xs, istd_g, li0):
                """qkv_vec_pair split into 4 closures, issued one per
                attention head so the DVE queue never sees a multi-us rope
                blob ahead of the latency-critical mask/ptacc adds."""
                st = {}
                return [
                    lambda: _vp_norm(t0, xs, istd_g, li0, 0, st),
                    lambda: _vp_norm(t0, xs, istd_g, li0, 1, st),
                    lambda: _vp_rope1(t0, st),
                    lambda: _vp_rope2(t0, st),
                ]

            def qkv_transposes(t, pool=None, tag=None):
                rot = rots.pop(t)
                li01 = t % 2
                # phase A: pso pool (idle there, so transposes never serialize
                # the qkv psum recycling); phase B: pab (idle there) so the
                # S-pipeline's pso ring stays private to attention
                pool = pool or pso
                tp = pool.tile([128, NH * 128], BF16, tag=tag or "so",
                               name=f"tp_{t}")
                ident = misc_sb[:, IDENT0:IDENT0 + 128]
                for h in range(NH):
                    c0 = li01 * CW + h * HALF
                    # two half-width transposes (PE weights APs must be 2D):
                    # rot half s lands on psum partitions [64s, 64s+64)
                    nc.tensor.transpose(
                        tp[0:64, h * 128:(h + 1) * 128],
                        rot[:, c0:c0 + HALF], ident, tile_position=(0, 0))
                    nc.tensor.transpose(
                        tp[64:128, h * 128:(h + 1) * 128],
                        rot[:, PW + c0:PW + c0 + HALF], ident,
                        tile_position=(0, 64))
                # one wide strided copy: slot h goes to qkt col h*T + t*128
                dst = qkt_sb[:].rearrange("p (h t) -> p h t", h=NH)[
                    :, :, t * 128:(t + 1) * 128]
                src = tp[:].rearrange("p (h d) -> p h d", h=NH)
                nc.vector.tensor_copy(dst, src)

            at_tiles = {}
            den_acc = {}
            den_ps_t = {}

            def _issue_den(g, h):
                """PE: one 512-col den matmul over the DVE-accumulated ptacc
                (replaces a per-key-block ones-matmul stream)."""
                ptacc = den_acc.pop((g, h))
                den_ps = pden.tile([1, QCW], F32, tag="den",
                                   name=f"den_{g}_{h}")
                nc.tensor.matmul(den_ps[0:1, :], ones_sb[:, 0:1], ptacc[:],
                                 start=True, stop=True)
                den_ps_t[(g, h)] = den_ps

            def _den_post(g, h):
                """DVE reciprocal + gpsimd partition broadcast for head h."""
                den_ps = den_ps_t.pop((g, h))
                den_r = wrk.tile([1, QCW], F32, tag="denr",
                                 name=f"denr_{g}_{h}")
                nc.vector.reciprocal_approx_fast(den_r[0:1, :],
                                                 den_ps[0:1, :])
                den_b = wrk.tile([128, QCW], F32, tag="denb",
                                 name=f"denb_{g}_{h}")
                nc.gpsimd.partition_broadcast(den_b[:], den_r[0:1, :])
                at, o_ps, _ = at_tiles[(g, h)]
                at_tiles[(g, h)] = (at, o_ps, den_b)

            def attn_chunk(g, fillers=()):
                """Causal attention for q-chunk g (512 q), heads sequential,
                S issued 3 key-blocks ahead of AV. den for head h-1 is
                computed inside head h's stream (matmul at kb2, recip+bcast
                at kb4, at-mul after the AV drain). fillers: one closure per
                head of deferred phase-A DVE work, spread between heads."""
                nkb = 4 * g + 4
                for h in range(HQL):
                    o_ps = pacc.tile([128, QCW], F32, tag="o", name=f"o_{g}_{h}")
                    ptacc = wrk.tile([128, QCW], BF16, tag="ptacc", bufs=2,
                                     name=f"ptacc_{g}_{h}")
                    pend = []
                    pt0 = [None]

                    def issue_s(kb):
                        r = kb - 4 * g
                        q0 = 128 * r if r > 0 else 0
                        s_ps = pso.tile([128, QCW], F32, tag="so",
                                        name=f"s_{g}_{h}_{kb}")
                        nc.tensor.matmul(
                            s_ps[:, q0:QCW],
                            qkt_sb[:, HQL * T + kb * 128:
                                   HQL * T + (kb + 1) * 128],
                            qkt_sb[:, h * T + g * QCW + q0:
                                   h * T + (g + 1) * QCW],
                            start=True, stop=True)
                        pt = wrk.tile([128, QCW], BF16, tag="pt", bufs=8,
                                      name=f"pt_{g}_{h}_{kb}")
                        nc.scalar.activation(pt[:, q0:QCW], s_ps[:, q0:QCW],
                                             AF.Exp, scale=SCALE)
                        if r >= 0:
                            nc.vector.tensor_mul(
                                pt[:, q0:q0 + 128], pt[:, q0:q0 + 128],
                                misc_sb[:, MASK0:MASK0 + 128])
                        if kb == 0:
                            if g == 0:
                                nc.vector.tensor_copy(ptacc[:], pt[:])
                            else:
                                pt0[0] = pt
                        elif pt0[0] is not None:
                            nc.vector.tensor_add(ptacc[:], pt0[0][:], pt[:])
                            pt0[0] = None
                        else:
                            nc.vector.tensor_add(ptacc[:, q0:QCW],
                                                 ptacc[:, q0:QCW],
                                                 pt[:, q0:QCW])
                        pend.append((kb, pt, q0))

                    def issue_av():
                        kb, pt, q0 = pend.pop(0)
                        nc.tensor.matmul(o_ps[:, q0:QCW],
                                         v_sb[:, kb * 128:(kb + 1) * 128],
                                         pt[:, q0:QCW], start=(kb == 0),
                                         stop=(kb == nkb - 1),
                                         skip_group_check=True)

                    # S issued in pairs, AV consumed in pairs 4 behind: the
                    # two AVs accumulate the same PSUM bank back-to-back,
                    # which pipelines without a bank-switch drain
                    # longer chunks defer the den matmul further so the DVE
                    # ptacc add chain (which trails the ACT exps) is done
                    den_kb2 = max(2, nkb // 2 - 2)
                    post_kb2 = min(den_kb2 + 2, nkb - 2)
                    for kb2 in range(0, nkb, 2):
                        issue_s(kb2)
                        issue_s(kb2 + 1)
                        if h > 0:
                            if kb2 == den_kb2:
                                _issue_den(g, h - 1)
                            if kb2 == post_kb2:
                                _den_post(g, h - 1)
                        if kb2 >= 4:
                            issue_av()
                            issue_av()
                    while pend:
                        issue_av()

                    at = wrk.tile([128, QCW], BF16, tag="at", bufs=8,
                                  name=f"at_{g}_{h}")
                    # deferred: at-mul for head h-1 issued during head h's
                    # stream so the gpsimd broadcast latency is hidden
                    if h > 0:
                        _flush_at(g, h - 1)
                    at_tiles[(g, h)] = (at, o_ps, None)
                    den_acc[(g, h)] = ptacc
                    if h < len(fillers):
                        fillers[h]()
                _issue_den(g, HQL - 1)
                _den_post(g, HQL - 1)
                _flush_at(g, HQL - 1)

            def _flush_at(g, h):
                at, o_ps, den_b = at_tiles[(g, h)]
                if o_ps is not None:
                    nc.vector.tensor_mul(at[:], o_ps[:], den_b[:])
                    at_tiles[(g, h)] = (at, None, None)

            def outproj_parts(g, nparts=4, evict_eng="dve"):
                # op tiles live in the pab pool: it is idle in phase B, so
                # outproj never contends with the attention S-pipeline's pso
                # ring (whose reuse waits on exp evictions)
                st = {'prev': None, 'pho': None}

                def part(lo, hi):
                    def run():
                        for ho in range(lo, hi):
                            op = pab.tile([128, QCW], F32, tag="ab",
                                          name=f"op_{g}_{ho}")
                            for f in range(HQL):
                                nc.tensor.matmul(
                                    op[:],
                                    wo_sb[:, f * HIDDEN + ho * 128:
                                          f * HIDDEN + ho * 128 + 128],
                                    at_tiles[(g, f)][0][:],
                                    start=(f == 0), stop=(f == HQL - 1))
                            if st['prev'] is not None:
                                _evict_osb(g, st['pho'], st['prev'],
                                           evict_eng)
                            st['prev'] = op
                            st['pho'] = ho
                        if hi == TT:
                            _evict_osb(g, TT - 1, st['prev'], evict_eng)
                    return run

                step = TT // nparts
                return [part(i * step, (i + 1) * step) for i in range(nparts)]

            def outproj(g):
                for p in outproj_parts(g, 1):
                    p()

            def _evict_osb(g, ho, op, eng="dve"):
                osb = wrk.tile([128, QCW], BF16, tag="osb", bufs=4,
                               name=f"osb_{g}_{ho}")
                # standalone outproj: DVE (keeps ACT pure-exp near attention)
                # filler-mode (inside attn3): ACT, because the DVE add chain
                # would delay the eviction and stall the pab ring
                if eng == "act":
                    nc.scalar.copy(osb[:], op[:])
                else:
                    nc.vector.tensor_copy(osb[:], op[:])
                # final chunk drains its output over two parallel queues so
                # the last eviction's DMA isn't queued behind 15 others
                eng = nc.gpsimd if (g == NQC - 1 and ho % 2 == 1) else nc.sync
                eng.dma_start(
                    out_tiled[ho][:, g * QCW:(g + 1) * QCW], osb[:])

            # ---- main schedule: single ACT table (Copy/Square/Exp) ---------
            # Phase A in groups of 4 tiles: [matmuls+evicts+ssq x4] ->
            # [batched Newton rsqrt] -> [norm+rope x4]; transposes of group G
            # are issued inside group G+1's matmul window. Group 3's vector
            # work is interleaved with the first attention chunks so it never
            # head-of-line-blocks the DVE queue at the phase boundary.
            xs = {}
            istd_gs = []
            for G in range(4):
                istd_g = wrk.tile([128, 4 * NH], F32, tag="istdg",
                                  name=f"istdg_{G}")
                istd_gs.append(istd_g)
                for li, t in enumerate(range(4 * G, 4 * G + 4)):
                    qkv_mm_tile(t, xs, istd_g, li)
                    if t - 4 in rots:
                        qkv_transposes(t - 4)
                group_rsqrt(istd_g, G)
                if G < 3:
                    qkv_vec_pair(4 * G, xs, istd_g, 0)
                    qkv_vec_pair(4 * G + 2, xs, istd_g, 2)
            # Attention (chunk g needs only q/k tiles <= 4g+3) interleaved
            # with group 3's deferred vector work (as per-head fillers) and
            # out-projections.
            attn_chunk(0)
            attn_chunk(1, fillers=vec_pair_fillers(12, xs, istd_gs[3], 0))
            outproj(0)
            attn_chunk(2, fillers=vec_pair_fillers(14, xs, istd_gs[3], 2))
            for t in range(12, 16):
                qkv_transposes(t, pool=pab, tag="ab")
            outproj(1)
            attn_chunk(3)
            outproj(2)
            outproj(NQC - 1)

    nc.compile()
    return nc


def _pack_rows(a):
    """[N*128, M] -> [128, N*M] SBUF image (partition-major k-tiles)."""
    n = a.shape[0] // 128
    return np.ascontiguousarray(
        a.reshape(n, 128, a.shape[1]).transpose(1, 0, 2).reshape(128, -1))


def _pack_ht(hsT):
    """hidden^T [2048, 2048] -> [128, (t_tile, kt, 128)] bf16."""
    # hsT[kt*128+p, tile*128+c] -> img[p, ((tile*KT)+kt)*128 + c]
    a = hsT.reshape(KT, 128, TT, 128)          # [kt, p, tile, c]
    a = a.transpose(1, 2, 0, 3)                # [p, tile, kt, c]
    return np.ascontiguousarray(a.reshape(128, -1))


def _cos_sin(positions):
    j = np.arange(HALF, dtype=np.float32)
    inv_freq = (np.float32(THETA) ** (-j / np.float32(HALF))).astype(np.float32)
    pos = positions.astype(np.float32)
    freqs3 = pos[:, :, None] * inv_freq[None, None, :]      # [3, T, HALF] f32
    sel = np.zeros(HALF, dtype=np.int64)
    sel[MROPE[0]:MROPE[0] + MROPE[1]] = 1
    sel[MROPE[0] + MROPE[1]:] = 2
    freqs = freqs3[sel, :, np.arange(HALF)].T               # [T, HALF]
    freqs = np.ascontiguousarray(freqs.astype(np.float32))
    return np.cos(freqs).astype(np.float32), np.sin(freqs).astype(np.float32)


def _prep_inputs(hidden_states, positions, w_qkv, w_o, q_norm_w, k_norm_w):
    ht = _pack_ht(np.ascontiguousarray(hidden_states.T).astype(bfloat16))
    cos, sin = _cos_sin(positions)
    cos_p = _pack_rows(np.tile(cos, (1, NH)).astype(bfloat16))
    sin_p = _pack_rows(np.tile(sin, (1, NH)).astype(bfloat16))
    qnw = np.tile(np.asarray(q_norm_w, np.float32)[None, :], (128, 1)
                  ).astype(bfloat16)
    knw = np.tile(np.asarray(k_norm_w, np.float32)[None, :], (128, 1)
                  ).astype(bfloat16)
    # diag mask: keys on partitions p, q offset j in the 128-wide diag block:
    # visible iff j >= p
    mask = (np.arange(128)[None, :] >= np.arange(128)[:, None]).astype(bfloat16)
    ident = np.eye(128, dtype=bfloat16)
    misc = np.concatenate([cos_p, sin_p, qnw, knw, mask, ident],
                          axis=1).astype(bfloat16)
    misc = np.ascontiguousarray(misc)

    in_maps = []
    for i in range(N_CORES):
        q0 = HQL * i * D
        wq = w_qkv[:, q0: q0 + HQL * D]
        wk = w_qkv[:, HQ * D + i * D: HQ * D + (i + 1) * D]
        wv = w_qkv[:, (HQ + HK) * D + i * D: (HQ + HK) * D + (i + 1) * D]
        wqkv_i = np.concatenate([wq, wk, wv], axis=1).astype(bfloat16)
        wo_i = w_o[HQL * i * D: HQL * (i + 1) * D, :].astype(bfloat16)
        in_maps.append({
            "ht": ht,
            "wqkv": _pack_rows(wqkv_i),
            "wo": _pack_rows(wo_i),
            "misc": misc,
        })
    return in_maps


LAST_RESULTS = None


def kernel(**inputs):
    global LAST_RESULTS
    if "nc" not in _CACHED:
        _CACHED["nc"] = _build()
    nc = _CACHED["nc"]
    in_maps = _prep_inputs(**{k: np.asarray(v) for k, v in inputs.items()})
    trace = bool(os.environ.get("BASS_TRACE"))
    res = run_bass_kernel_spmd(nc, in_maps, core_ids=list(range(N_CORES)),
                               trace=trace)
    LAST_RESULTS = res
    acc = np.zeros((HIDDEN, T), dtype=np.float32)
    for i in range(N_CORES):
        acc += res.results[i]["out"].astype(np.float32)
    return np.ascontiguousarray(acc.T)

